# revision 1
# baseline (speedup 1.0000x reference)
"""Data-parallel Trainium2 kernel for nn_AttentionModel_55198919688262.

Strategy (per sharding hint): shard the batch dim (B=128) across the 8
NeuronCores (16 rows each), replicate all weights. The sequential scans
over time run locally per device. Inputs are taken whole; sharding and
the final gather happen inside this function.
"""

import numpy as np

B, T, V, D, H = 128, 64, 30000, 300, 512
NEG = -10000.0
FORGET_BIAS = 1.0
N_CORES = 8

_COMPILED = {}


def _get_impl():
    if "impl" in _COMPILED:
        return _COMPILED["impl"]

    import jax
    import jax.numpy as jnp

    def lstm(x, seqlen, W, b):
        xT = jnp.swapaxes(x, 0, 1)  # [T,Bl,Din]
        valid = (jnp.arange(T)[:, None] < seqlen[None, :]).astype(x.dtype)[:, :, None]
        c0 = jnp.zeros((x.shape[0], H), x.dtype)
        h0 = jnp.zeros((x.shape[0], H), x.dtype)

        def step(carry, inp):
            c, h = carry
            xt, v = inp
            g = jnp.concatenate([xt, h], axis=-1) @ W + b
            i, j, f, o = jnp.split(g, 4, axis=-1)
            c_new = c * jax.nn.sigmoid(f + FORGET_BIAS) + jax.nn.sigmoid(i) * jnp.tanh(j)
            h_new = jnp.tanh(c_new) * jax.nn.sigmoid(o)
            c = v * c_new + (1.0 - v) * c
            h = v * h_new + (1.0 - v) * h
            return (c, h), v * h_new

        (c, h), Y = jax.lax.scan(step, (c0, h0), (xT, valid))
        return jnp.swapaxes(Y, 0, 1), h

    def asym(e1, s1, e2, s2, mask1, W1, b1, W2, b2, Wy, Wh, Wr, wv, Wt, Wp, Wx):
        Y, _ = lstm(e1, s1, W1, b1)
        Y2, last_h = lstm(e2, s2, W2, b2)
        WyY = Y @ Wy
        Y2T = jnp.swapaxes(Y2, 0, 1)
        r0 = jnp.zeros_like(last_h)

        def step(r, h_t):
            M = jnp.tanh(WyY + (h_t @ Wh + r @ Wr)[:, None, :])
            alpha = jax.nn.softmax(jnp.sum(M * wv, axis=2) + mask1, axis=1)
            Y_alpha = jnp.einsum('bth,bt->bh', Y, alpha)
            r_new = Y_alpha + jnp.tanh(r @ Wt)
            return r_new, r_new

        _, r_seq = jax.lax.scan(step, r0, Y2T)
        oh = jax.nn.one_hot(s2 - 1, T, dtype=r_seq.dtype)
        r_L = jnp.einsum('tbh,bt->bh', r_seq, oh)
        return jnp.tanh(r_L @ Wp + last_h @ Wx)

    def shard_fn(tokens1, tokens2, seqlen1, seqlen2, emb, W1, b1, W2, b2,
                 Wy, Wh, Wr, wv, Wt, Wp, Wx, U, bU):
        e1 = emb[tokens1]
        e2 = emb[tokens2]
        t_idx = jnp.arange(T)[None, :]
        mask1 = jnp.where(t_idx < seqlen1[:, None], 0.0, NEG).astype(jnp.float32)
        mask2 = jnp.where(t_idx < seqlen2[:, None], 0.0, NEG).astype(jnp.float32)
        la = asym(e1, seqlen1, e2, seqlen2, mask1, W1, b1, W2, b2,
                  Wy, Wh, Wr, wv, Wt, Wp, Wx)
        lb = asym(e2, seqlen2, e1, seqlen1, mask2, W1, b1, W2, b2,
                  Wy, Wh, Wr, wv, Wt, Wp, Wx)
        return (la + lb) @ U + bU

    devices = jax.devices()[:N_CORES]
    # batch args sharded on axis 0; weights replicated
    impl = jax.pmap(
        shard_fn,
        in_axes=(0, 0, 0, 0) + (None,) * 14,
        devices=devices,
    )
    _COMPILED["impl"] = impl
    return impl


def kernel(tokens1, tokens2, seqlen1, seqlen2, emb, W1, b1, W2, b2,
           Wy, Wh, Wr, wv, Wt, Wp, Wx, U, bU):
    impl = _get_impl()
    bl = B // N_CORES

    def sh(a):
        return np.ascontiguousarray(a.reshape(N_CORES, bl, *a.shape[1:]))

    out = impl(sh(np.asarray(tokens1)), sh(np.asarray(tokens2)),
               sh(np.asarray(seqlen1)), sh(np.asarray(seqlen2)),
               np.asarray(emb, np.float32), np.asarray(W1, np.float32),
               np.asarray(b1, np.float32), np.asarray(W2, np.float32),
               np.asarray(b2, np.float32), np.asarray(Wy, np.float32),
               np.asarray(Wh, np.float32), np.asarray(Wr, np.float32),
               np.asarray(wv, np.float32), np.asarray(Wt, np.float32),
               np.asarray(Wp, np.float32), np.asarray(Wx, np.float32),
               np.asarray(U, np.float32), np.asarray(bU, np.float32))
    out = np.asarray(out)
    return out.reshape(B, out.shape[-1]).astype(np.float32)
